# revision 28
# baseline (speedup 1.0000x reference)
"""Trainium2 Bass kernel for nn_ConvDY2d (dynamic-weight 3x3 conv, CondConv-style).

Reference computation (B=16, C=O=256, H=W=64, K=4 mixing kernels):
  attn  = softmax(MLP(global_avg_pool(x)) / 30)            # [B, 4]
  w_mix = einsum('bk,koihw->boihw', attn, w_dyn)           # per-sample 3x3 conv kernel
  out[b] = conv2d(x[b], w_mix[b], padding=1)

Strategy: data-parallel over batch, 2 samples per NeuronCore across 8 cores.
Per core, the conv is an implicit GEMM: for each (out-channel block, 8-row
group) a [128, 512] PSUM tile accumulates 18 bf16 matmuls (2 c-blocks x
9 taps) whose rhs are contiguous 512-element slices of a row-padded input
image ([128c, 4226]).  Column wrap-around at row edges is fixed up by
subtracting border corrections computed from compacted border-column gathers.

Weight mixing runs in bf16 mean+delta form (wm = wbar + sum_k gamma_k wdiff_k,
gamma = attn - 1/4, wdiff in fp8), 3 DVE passes chunked by tap position so
conv matmuls start while weights still stream in.  DMA is spread over the
three HWDGE queues with dep-chained issue order; vector/scalar compute is
also dep-chained in emission order so the tile scheduler cannot starve the
critical attention->mix chain.  Outputs are bf16 (host upcasts).
"""

import sys

if "/opt/trn_rl_repo" not in sys.path:
    sys.path.insert(0, "/opt/trn_rl_repo")

import numpy as np

B, C, H, W = 16, 256, 64, 64
O, K, KS = 256, 4, 3
MID = C // 4
INV_DELTA = 1.0 / 30.0
NCORES = 8
NB = B // NCORES            # samples per core
NPOS = KS * KS              # 9 taps
FPAD = 1 + 66 * W + 1       # padded image free size: 4226
ROW0 = 65                   # flat offset of input row 0 (= 1 + 1*64)

_CACHE = {}


def _build_nc():
    import concourse.bacc as bacc
    import concourse.tile as tile
    from concourse import mybir
    from concourse.tile_rust import add_dep_helper

    f32 = mybir.dt.float32
    AX = mybir.AxisListType
    ALU = mybir.AluOpType
    ACTF = mybir.ActivationFunctionType

    nc = bacc.Bacc(target_bir_lowering=False, debug=False)

    bf16 = mybir.dt.bfloat16
    f8 = mybir.dt.float8e4

    x_d = nc.dram_tensor("x", [NB, C, H, W], bf16, kind="ExternalInput").ap()
    wbar_d = nc.dram_tensor("wbar", [C, NPOS * O], bf16, kind="ExternalInput").ap()
    wdiff_d = nc.dram_tensor("wdiff", [3, C, NPOS * O], f8, kind="ExternalInput").ap()
    fc1wT_d = nc.dram_tensor("fc1wT", [C, MID], f32, kind="ExternalInput").ap()
    fc1b_d = nc.dram_tensor("fc1b", [MID, 1], f32, kind="ExternalInput").ap()
    fc2aug_d = nc.dram_tensor("fc2aug", [MID + 1, K], f32, kind="ExternalInput").ap()
    out_d = nc.dram_tensor("out", [NB, O, H, W], bf16, kind="ExternalOutput").ap()

    # per-queue issue-order chains + per-engine compute-order chains
    chains = {}

    def chained(key, ins):
        prev = chains.get(key)
        if prev is not None:
            add_dep_helper(ins.ins, prev.ins, sync=False, reason=f"{key} order")
        chains[key] = ins
        return ins

    with tile.TileContext(nc) as tc:
        with (
            tc.tile_pool(name="consts", bufs=1) as constp,
            tc.tile_pool(name="wdyn", bufs=1) as wdynp,
            tc.tile_pool(name="wmix", bufs=1) as wmixp,
            tc.tile_pool(name="xpad", bufs=1) as xpadp,
            tc.tile_pool(name="osb", bufs=6) as osbp,
            tc.tile_pool(name="convps", bufs=4, space="PSUM") as convps,
            tc.tile_pool(name="corrps", bufs=2, space="PSUM") as corrps,
            tc.tile_pool(name="smallps", bufs=1, space="PSUM") as smallps,
        ):
            def qdma(qname, eng, dst, src):
                # scalar-queue DMAs share the Activation engine with compute:
                # keep them in ONE total order so the scheduler cannot slot
                # x1 transfers ahead of critical x0 chunks (v5 failure mode)
                key = "act" if qname == "scalar" else qname
                return chained(key, eng.dma_start(dst, src))

            def vop(f, *a, **kw):
                return chained("dve", f(*a, **kw))

            def sop(f, *a, **kw):
                return chained("act", f(*a, **kw))

            # ---------------- x[0]: pad memsets + interior chunks ------------------
            # 8 chunks of 8 rows per c-block, spread over all three DMA queues
            xpad = [[None, None] for _ in range(NB)]
            for cb in range(2):
                t = xpadp.tile([128, FPAD], bf16, tag=f"xpad0{cb}", name=f"xpad0{cb}")
                nc.gpsimd.memset(t[:, 0:ROW0], 0.0)
                nc.gpsimd.memset(t[:, ROW0 + H * W : FPAD], 0.0)
                xpad[0][cb] = t

            QENG = {"sync": nc.sync, "scalar": nc.scalar, "gpsimd": nc.gpsimd}
            # 4 chunks of 16 rows per c-block (2KB/partition = full ring rate);
            # cb0 on a clean sync ring, cb1 on a clean scalar ring
            for h in range(4):
                for cb, qn in ((0, "sync"), (1, "scalar")):
                    qdma(
                        qn, QENG[qn],
                        xpad[0][cb][:, ROW0 + h * 16 * W : ROW0 + (h + 1) * 16 * W],
                        x_d[0, cb * 128 : (cb + 1) * 128, h * 16 : (h + 1) * 16, :]
                        .rearrange("c h w -> c (h w)"),
                    )

            # ---------------- small consts (gpsimd queue, tiny) --------------------
            fc1wT_sb = constp.tile([128, 2 * MID], f32, tag="fc1w", name="fc1wT_sb")
            for cb in range(2):
                qdma("gpsimd", nc.gpsimd,
                     fc1wT_sb[:, cb * MID : (cb + 1) * MID],
                     fc1wT_d[cb * 128 : (cb + 1) * 128, :])
            fc1b_sb = constp.tile([MID, 1], f32, tag="fc1b", name="fc1b_sb")
            qdma("gpsimd", nc.gpsimd, fc1b_sb, fc1b_d)
            fc2aug_sb = constp.tile([MID + 1, K], f32, tag="fc2", name="fc2aug_sb")
            qdma("gpsimd", nc.gpsimd, fc2aug_sb, fc2aug_d)

            # ---------------- mixing-source weights --------------------------------
            # 3 chunks per tensor (tap pos 0 | 1-4 | 5-8); cb0 on sync, cb1 gpsimd
            wbar_sb = [None, None]
            wdiff_sb = [[None, None] for _ in range(3)]
            for cb in range(2):
                wbar_sb[cb] = wdynp.tile([128, NPOS * O], bf16, tag=f"wb{cb}", name=f"wbar{cb}")
                for j in range(3):
                    wdiff_sb[j][cb] = wdynp.tile(
                        [128, NPOS * O], f8, tag=f"wd{j}{cb}", name=f"wdiff{j}{cb}"
                    )
            WCHUNKS = ((0, O), (O, 5 * O), (5 * O, NPOS * O))
            # wbar pos-chunked; each fp8 wdiff as ONE full-width DMA (2.25KB/
            # partition keeps the ring at full rate).  cb0 on sync (behind x0
            # cb0), cb1 on gpsimd (behind consts only).
            for cb, qn in ((0, "sync"), (1, "gpsimd")):
                rows = slice(cb * 128, (cb + 1) * 128)
                qdma(qn, QENG[qn], wbar_sb[cb][:, 0:O], wbar_d[rows, 0:O])
                for j in range(3):
                    qdma(qn, QENG[qn], wdiff_sb[j][cb], wdiff_d[j, rows, :])
                qdma(qn, QENG[qn], wbar_sb[cb][:, O : 5 * O], wbar_d[rows, O : 5 * O])
                qdma(qn, QENG[qn], wbar_sb[cb][:, 5 * O :], wbar_d[rows, 5 * O :])

            # ---------------- x[1] load (scalar queue, after x0 cb1) ---------------
            def load_x1():
                for cb in range(2):
                    t = xpadp.tile([128, FPAD], bf16, tag=f"xpad1{cb}", name=f"xpad1{cb}")
                    nc.gpsimd.memset(t[:, 0:ROW0], 0.0)
                    nc.gpsimd.memset(t[:, ROW0 + H * W : FPAD], 0.0)
                    xpad[1][cb] = t
                for h in range(2):
                    for cb in range(2):
                        qdma(
                            "scalar", nc.scalar,
                            xpad[1][cb][:, ROW0 + h * 32 * W : ROW0 + (h + 1) * 32 * W],
                            x_d[1, cb * 128 : (cb + 1) * 128, h * 32 : (h + 1) * 32, :]
                            .rearrange("c h w -> c (h w)"),
                        )

            ones_sb = constp.tile([1, 128], f32, tag="ones", name="ones_sb")
            vop(nc.vector.memset, ones_sb, 1.0)
            act_dummy = constp.tile([128, 1024], bf16, tag="actdum", name="act_dummy")

            # ---------------- per-sample attention -> mixed weights ----------------
            wmix = [[None, None] for _ in range(NB)]
            prev_mix = [None]

            def attn_and_mix(b, nchunk):
                # global sum pool: cb0 partial reduces on DVE, cb1 on ACT for
                # b=0 (parallel chase); all-DVE for b=1 (ACT drains conv PSUM)
                pooled = [None, None]
                chunk = H * W // nchunk
                pp = [None, None]
                for cb in range(2):
                    pp[cb] = constp.tile([128, nchunk], f32, tag=f"pp{b}{cb}", name=f"pp{b}{cb}")
                for h in range(nchunk):
                    for cb in range(2):
                        src = xpad[b][cb][:, ROW0 + h * chunk : ROW0 + (h + 1) * chunk]
                        if cb == 0 or b == 1:
                            vop(nc.vector.reduce_sum, pp[cb][:, h : h + 1], src, AX.X)
                        else:
                            sop(nc.scalar.activation, act_dummy[:, 0:chunk], src,
                                ACTF.Copy, accum_out=pp[cb][:, h : h + 1])
                for cb in range(2):
                    p = constp.tile([128, 1], f32, tag=f"pool{b}{cb}", name=f"pooled{b}{cb}")
                    vop(nc.vector.reduce_sum, p, pp[cb], AX.X)
                    pooled[cb] = p

                hid_ps = smallps.tile([MID, 1], f32, tag="small", name=f"hid_ps{b}")
                for cb in range(2):
                    nc.tensor.matmul(
                        hid_ps,
                        fc1wT_sb[:, cb * MID : (cb + 1) * MID],
                        pooled[cb],
                        start=(cb == 0),
                        stop=(cb == 1),
                    )
                # hid = relu(hid_ps + fc1b) on DVE; aug row = 1 for fc2 bias
                hid_sb = constp.tile([MID + 1, 1], f32, tag=f"hid{b}", name=f"hid_sb{b}")
                vop(nc.vector.memset, hid_sb[MID : MID + 1, :], 1.0)
                vop(nc.vector.tensor_scalar,
                    hid_sb[0:MID, :], hid_ps, fc1b_sb, 0.0, op0=ALU.add, op1=ALU.max)

                lg_ps = smallps.tile([1, K], f32, tag="small", name=f"lg_ps{b}")
                nc.tensor.matmul(lg_ps, hid_sb, fc2aug_sb, start=True, stop=True)

                # gamma_k = exp(lg_k)/sum(exp) - 1/4 (softmax norm folded in)
                ex = constp.tile([1, K], f32, tag=f"ex{b}", name=f"ex{b}")
                sm = constp.tile([1, 1], f32, tag=f"sm{b}", name=f"sm{b}")
                sop(nc.scalar.activation, ex, lg_ps, ACTF.Exp, accum_out=sm)
                rc = constp.tile([1, 1], f32, tag=f"rc{b}", name=f"rc{b}")
                vop(nc.vector.reciprocal, rc, sm)
                gam = constp.tile([1, K], f32, tag=f"at{b}", name=f"gam{b}")
                vop(nc.vector.tensor_scalar,
                    gam, ex, rc, 0.25, op0=ALU.mult, op1=ALU.subtract)
                # broadcast gamma to 128 partitions via rank-1 PE matmul
                gam_bc = smallps.tile([128, K], f32, tag="gbc", name=f"gam_bc{b}")
                nc.tensor.matmul(gam_bc, ones_sb, gam, start=True, stop=True)

                # mixed weights, bf16 3-pass delta form, chunked by tap position
                for cb in range(2):
                    wmix[b][cb] = wmixp.tile(
                        [128, NPOS * O], bf16, tag=f"wm{b}{cb}", name=f"wmix{b}{cb}"
                    )
                for lo, hi in WCHUNKS:
                    for cb in range(2):
                        wm = wmix[b][cb]
                        vop(nc.vector.scalar_tensor_tensor,
                            wm[:, lo:hi], wdiff_sb[0][cb][:, lo:hi], gam_bc[:, 1:2],
                            wbar_sb[cb][:, lo:hi], op0=ALU.mult, op1=ALU.add)
                        for j in (1, 2):
                            vop(nc.vector.scalar_tensor_tensor,
                                wm[:, lo:hi], wdiff_sb[j][cb][:, lo:hi],
                                gam_bc[:, j + 1 : j + 2], wm[:, lo:hi],
                                op0=ALU.mult, op1=ALU.add)

            attn_and_mix(0, 4)
            load_x1()

            # ---------------- border-column gathers for corrections ----------------
            gtile = [[None, None] for _ in range(NB)]

            def gather_borders(b):
                for cb in range(2):
                    g = constp.tile([128, 132], bf16, tag=f"g{b}{cb}", name=f"g{b}{cb}")
                    vop(nc.vector.tensor_scalar_add,
                        g[:, 0:66], xpad[b][cb][:, 0 : 65 * W + 1 : W], 0.0)
                    vop(nc.vector.tensor_scalar_add,
                        g[:, 66:132], xpad[b][cb][:, ROW0 : ROW0 + 65 * W + 1 : W], 0.0)
                    gtile[b][cb] = g

            # ---------------- main conv ----------------
            def wsl(b, cb, pos, ob):
                off = pos * O + ob * 128
                return wmix[b][cb][:, off : off + 128]

            def emit_corr(b, ob):
                corr = corrps.tile([128, 128], f32, tag="corr", name=f"corr{b}{ob}")
                for side, dxv in ((0, 0), (1, 2)):
                    i = 0
                    for cb in range(2):
                        for dy in range(KS):
                            g0 = side * 66 + dy
                            nc.tensor.matmul(
                                corr[:, side * 64 : side * 64 + 64],
                                wsl(b, cb, dy * KS + dxv, ob),
                                gtile[b][cb][:, g0 : g0 + 64],
                                start=(i == 0),
                                stop=(i == 5),
                            )
                            i += 1
                return corr

            POSCHUNK = ((0,), (1, 2, 3, 4), (5, 6, 7, 8))

            def emit_conv_group(b, ob, rg):
                y0 = rg * 8
                cps = convps.tile([128, 512], f32, tag="conv", name=f"cps{b}{ob}{rg}")
                i = 0
                for pc in POSCHUNK:
                    for cb in range(2):
                        for pos in pc:
                            dy, dx = divmod(pos, KS)
                            s = (y0 + dy) * W + dx
                            nc.tensor.matmul(
                                cps,
                                wsl(b, cb, pos, ob),
                                xpad[b][cb][:, s : s + 512],
                                start=(i == 0),
                                stop=(i == 17),
                            )
                            i += 1
                osb = osbp.tile([128, 512], bf16, tag="osb", name=f"osb{b}{ob}{rg}")
                sop(nc.scalar.copy, osb, cps)
                return osb

            def emit_fix_and_out(b, ob, rg, osb, corr, last):
                y0 = rg * 8
                ov = osb.rearrange("m (y x) -> m y x", x=W)[:, :, 0 : W : W - 1]
                cv = corr.rearrange("m (s y) -> m y s", s=2)[:, y0 : y0 + 8, :]
                vop(nc.vector.tensor_sub, ov, ov, cv)
                dst = out_d[b, ob * 128 : (ob + 1) * 128, y0 : y0 + 8, :]
                src = osb.rearrange("m (y x) -> m y x", x=W)
                if last:
                    qdma("scalar", nc.scalar, dst[0:64], src[0:64])
                    qdma("gpsimd", nc.gpsimd, dst[64:128], src[64:128])
                elif rg % 2 == 0:
                    qdma("gpsimd", nc.gpsimd, dst, src)
                else:
                    qdma("scalar", nc.scalar, dst, src)

            def conv_section(b, ob, corr_after=2):
                pend = []
                for rg in range(corr_after):
                    pend.append((rg, emit_conv_group(b, ob, rg)))
                corr = emit_corr(b, ob)
                for rg, osb in pend:
                    emit_fix_and_out(b, ob, rg, osb, corr, last=False)
                for rg in range(corr_after, 8):
                    osb = emit_conv_group(b, ob, rg)
                    emit_fix_and_out(
                        b, ob, rg, osb, corr,
                        last=(b == NB - 1 and ob == 1 and rg == 7),
                    )

            gather_borders(0)
            conv_section(0, 0, corr_after=3)
            # sample 1's attention overlaps sample 0's conv
            attn_and_mix(1, 2)
            conv_section(0, 1)
            gather_borders(1)
            conv_section(1, 0)
            conv_section(1, 1)

    nc.compile()
    return nc


def get_nc():
    if "nc" not in _CACHE:
        _CACHE["nc"] = _build_nc()
    return _CACHE["nc"]


def prep_inputs(x, w_dyn, fc1_w, fc1_b, fc2_w, fc2_b):
    """Host-side layout prep + batch sharding -> per-core input maps."""
    import ml_dtypes

    bf16 = ml_dtypes.bfloat16
    wt = np.transpose(np.asarray(w_dyn, np.float32), (0, 2, 3, 4, 1)).reshape(
        K, C, NPOS * O
    )
    wbar = np.ascontiguousarray(wt.mean(axis=0)).astype(bf16)
    wdiff = np.ascontiguousarray(wt[1:] - wt[0:1]).astype(ml_dtypes.float8_e4m3)
    fc1wT = np.ascontiguousarray(np.asarray(fc1_w, np.float32).T) / float(H * W)
    fc1b = np.ascontiguousarray(np.asarray(fc1_b, np.float32).reshape(MID, 1))
    fc2aug = np.ascontiguousarray(
        np.vstack([np.asarray(fc2_w, np.float32).T, np.asarray(fc2_b, np.float32)[None, :]])
        * INV_DELTA
    )
    x = np.asarray(x, np.float32).astype(bf16)
    in_maps = []
    for core in range(NCORES):
        in_maps.append(
            {
                "x": np.ascontiguousarray(x[core * NB : (core + 1) * NB]),
                "wbar": wbar,
                "wdiff": wdiff,
                "fc1wT": fc1wT,
                "fc1b": fc1b,
                "fc2aug": fc2aug,
            }
        )
    return in_maps


def kernel(x, w_dyn, fc1_w, fc1_b, fc2_w, fc2_b):
    from concourse.bass_utils import run_bass_kernel_spmd

    nc = get_nc()
    in_maps = prep_inputs(x, w_dyn, fc1_w, fc1_b, fc2_w, fc2_b)
    res = run_bass_kernel_spmd(nc, in_maps, core_ids=list(range(NCORES)))
    return np.concatenate(
        [r["out"].astype(np.float32) for r in res.results], axis=0
    )
